# revision 10
# baseline (speedup 1.0000x reference)
# Trainium2 Bass kernel for nn_Decorrelation: out[:,v] = x[:,v] + sum_{c<v} lambda_{v,c}(x_c)*x_c
#
# Math: lambda_{v,c}(x) = B(x) @ params[:, pair(v,c)] with B the 16-fn cubic
# B-spline basis on clamped knots over [-6, 6].  The whole update is recast as
# one feature contraction out = x + G @ W where, per covariate c, the features
#     psi = { x, x^2, x^3, x^4 } + { p3_i = r_i^3 } + { p4_i = r_i^4 },
#     r_i = relu(s_i*(x - e_i))
# (near-side-facing truncated powers, s_i = -1 for e_i<=0 else +1) span
# x*span(B); W = C @ params is built on host from a float64 change-of-basis C.
# Sharding: data-parallel over samples, 16384 rows per core, 8 cores.
#
# Per 2048-sample supertile (per core):
#   DMA in [128,512] -> PE transpose (4x 128x128) to (quarter,var)-major ->
#   DVE clip -> PE replicate knot rows (12 slots/var, 3 chunks, row-tiled
#   across quarters) -> ACT relu -> ACT square -> DVE cube (TT) / ACT+GPSIMD
#   quartic -> power planes x^2..x^4 (ACT/DVE on quarter-stacked tiles,
#   packed per quarter by SBUF DMA) -> PE contract (7 K=128 matmuls per
#   quarter, col-tiled across quarters, fp32 accumulate in PSUM) -> ACT evac
#   -> PE transpose back -> DVE add x -> DMA out.
import numpy as np

V = 32
K = 16
DEG = 3
LO, HI = -5.0, 5.0
SPAN = 0.1 * (HI - LO)
lo, hi = LO - SPAN, HI + SPAN            # -6, 6
NB = K - DEG + 1                          # 14 breakpoints
INNER = np.linspace(lo, hi, NB)
EKNOT = INNER[1:-1]                       # 12 interior knots
KSGN = np.where(EKNOT <= 0, -1.0, 1.0)    # near-side facing
CHI = hi - 1e-6 * (hi - lo)
CLO = lo

N_TOTAL = 131072
NCORE = 8
NPER = N_TOTAL // NCORE                   # 16384
ST = 2048                                 # samples per supertile
FD = 512                                  # free dim of on-chip tiles

NSLOT = 12                                # knot slots, rows r = slot*32 + c
NROW = NSLOT * V                          # 384 rep rows
NCH = 3                                   # rep chunks of 128 rows
NCON = 7                                  # contract matmuls per quarter
CFD = NCH * FD                            # 1536: per-quarter feature free dim


def _bspline_basis(x):
    x = np.asarray(x, np.float64)
    t = np.concatenate([np.full(DEG, lo), INNER, np.full(DEG, hi)])
    x = np.clip(x, lo, hi - 1e-6 * (hi - lo))
    xb = x[:, None]
    B = ((xb >= t[None, :-1]) & (xb < t[None, 1:])).astype(np.float64)
    for r in range(1, DEG + 1):
        tl, tr = t[:-(r + 1)], t[r:-1]
        den1 = tr - tl
        left = np.where(den1 > 0, (xb - tl) / np.where(den1 > 0, den1, 1.0), 0.0)
        tl2, tr2 = t[1:-r], t[r + 1:]
        den2 = tr2 - tl2
        right = np.where(den2 > 0, (tr2 - xb) / np.where(den2 > 0, den2, 1.0), 0.0)
        B = left * B[:, :-1] + right * B[:, 1:]
    return B


_C_CACHE = None


def _fit_C():
    """C [28, 16] float64 s.t. x*B_k(x) = sum_m psi_m(x) * C[m, k],
    psi = [x, x^2, x^3, x^4, r_i^3 (12), r_i^4 (12)]."""
    global _C_CACHE
    if _C_CACHE is not None:
        return _C_CACHE
    g = np.linspace(lo, hi - 1e-5, 8001)
    cols = [g, g * g, g ** 3, g ** 4]
    for e, s in zip(EKNOT, KSGN):
        r = np.maximum(s * (g - e), 0.0)
        cols.append(r ** 3)
    for e, s in zip(EKNOT, KSGN):
        r = np.maximum(s * (g - e), 0.0)
        cols.append(r ** 4)
    Psi = np.stack(cols, axis=1)
    targ = g[:, None] * _bspline_basis(g)
    sc = np.abs(Psi).max(axis=0)
    C, _, _, _ = np.linalg.lstsq(Psi / sc, targ, rcond=None)
    C = C / sc[:, None]
    _C_CACHE = C
    return C


def _build_weights(params):
    """WG [128, 7*32] fp32 (chunk-major in free dim):
       chunk 0   : power features, row p*32+c -> x^(p+1) of var c
       chunks 1-3: r^3 features, row (i%4)*32+c for knot i in chunk i//4
       chunks 4-6: r^4 features, same layout
    """
    C = _fit_C()
    iv, ic = np.tril_indices(V, -1)
    P = np.zeros((K, V, V), np.float64)   # P[k, c, v]
    P[:, ic, iv] = params.astype(np.float64)
    CP = np.einsum("mk,kcv->mcv", C, P)   # [28, c, v]
    WG = np.zeros((NCON, 128, V), np.float64)
    for p in range(4):
        WG[0, p * 32:(p + 1) * 32, :] = CP[p]
    for i in range(12):
        j, sl = divmod(i, 4)
        WG[1 + j, sl * 32:(sl + 1) * 32, :] = CP[4 + i]
        WG[4 + j, sl * 32:(sl + 1) * 32, :] = CP[16 + i]
    WGt = WG.transpose(1, 0, 2).reshape(128, NCON * V)
    return np.ascontiguousarray(WGt, dtype=np.float32)


def _build_static_consts():
    # SREP replicated 4x vertically (rep matmul for quarter q uses
    # partitions 32q..32q+31 for both operands)
    srep1 = np.zeros((V, NROW), np.float32)
    for r in range(NROW):
        srep1[r % 32, r] = 1.0
    srep = np.tile(srep1, (4, 1))
    ks = np.zeros((128, NCH), np.float32)
    kb = np.zeros((128, NCH), np.float32)
    for j in range(NCH):
        for r in range(128):
            i = (r // 32) + 4 * j         # knot index
            ks[r, j] = KSGN[i]
            kb[r, j] = -KSGN[i] * EKNOT[i]
    ident = np.eye(128, dtype=np.float32)
    return srep, ks, kb, ident


_PROG_CACHE = {}


def _build_program(nper=NPER, ncore=NCORE, p4_act=2, rep_rowtile=True,
                   con_coltile=True, pipe_ahead=1):
    """p4_act: how many of the 4 per-quarter p4 (quartic) squares go on ACT;
    the rest run on GPSIMD as s2*s2."""
    key = (nper, ncore, p4_act, rep_rowtile, con_coltile, pipe_ahead)
    if key in _PROG_CACHE:
        return _PROG_CACHE[key]

    import concourse.mybir as mybir
    import concourse.tile as tile
    from concourse import bacc

    f32 = mybir.dt.float32
    Alu = mybir.AluOpType
    Act = mybir.ActivationFunctionType

    nst = nper // ST
    nc = bacc.Bacc("TRN2", target_bir_lowering=False, debug=False,
                   num_devices=ncore)
    x_d = nc.dram_tensor("input", [nper, V], f32, kind="ExternalInput")
    srep_d = nc.dram_tensor("SREP", [128, NROW], f32, kind="ExternalInput")
    ks_d = nc.dram_tensor("KS", [128, NCH], f32, kind="ExternalInput")
    kb_d = nc.dram_tensor("KB", [128, NCH], f32, kind="ExternalInput")
    wg_d = nc.dram_tensor("WG", [128, NCON * V], f32, kind="ExternalInput")
    id_d = nc.dram_tensor("IDENT", [128, 128], f32, kind="ExternalInput")
    y_d = nc.dram_tensor("output", [nper, V], f32, kind="ExternalOutput")

    # x[st*2048 + p*16 + s, v] <-> tile[p, s*32+v] (2KB contiguous per row)
    x_re = x_d.ap().rearrange("(st p s) v -> st p (s v)", p=128, s=16)
    y_re = y_d.ap().rearrange("(st p s) v -> st p (s v)", p=128, s=16)

    with tile.TileContext(nc) as tc:
        with (
            tc.tile_pool(name="const", bufs=1) as cpool,
            tc.tile_pool(name="io", bufs=3) as iopool,
            tc.tile_pool(name="xt", bufs=2) as xtpool,
            tc.tile_pool(name="feat", bufs=2) as fpool,
            tc.tile_pool(name="ps_head", bufs=2, space="PSUM") as ps_head,
            tc.tile_pool(name="ps_rep", bufs=4, space="PSUM") as ps_rep,
            tc.tile_pool(name="ps_tail", bufs=1, space="PSUM") as ps_tail,
        ):
            srep_t = cpool.tile([128, NROW], f32)
            ks_t = cpool.tile([128, NCH], f32)
            kb_t = cpool.tile([128, NCH], f32)
            wg_t = cpool.tile([128, NCON * V], f32)
            id_t = cpool.tile([128, 128], f32)
            for t, d in [(srep_t, srep_d), (ks_t, ks_d), (kb_t, kb_d),
                         (wg_t, wg_d), (id_t, id_d)]:
                nc.sync.dma_start(out=t[:], in_=d.ap())

            state = {}

            def head(st):
                x_nat = iopool.tile([128, FD], f32, tag="x_nat")
                nc.sync.dma_start(out=x_nat[:], in_=x_re[st])
                xt4_ps = ps_head.tile([128, FD], f32, tag="xt4")
                for g in range(4):
                    nc.tensor.transpose(
                        xt4_ps[:, g * 128:(g + 1) * 128],
                        x_nat[:, g * 128:(g + 1) * 128],
                        id_t[:],
                    )
                xt4 = xtpool.tile([128, FD], f32, tag="xt4sb")
                nc.vector.tensor_scalar(xt4[:], xt4_ps[:], CHI, CLO,
                                        op0=Alu.min, op1=Alu.max)
                # replicate knot rows; y written per-chunk into one big tile
                xreps = []
                for j in range(NCH):
                    for q in range(4):
                        xr = ps_rep.tile([128, FD], f32, tag="xrep")
                        nc.tensor.matmul(
                            xr[:],
                            srep_t[32 * q:32 * q + 32, j * 128:(j + 1) * 128],
                            xt4[32 * q:32 * q + 32, :],
                            start=True, stop=True,
                            tile_position=(32 * q, 0) if rep_rowtile else None,
                        )
                        xreps.append((j, q, xr))
                ybig = []
                for q in range(4):
                    yb = fpool.tile([128, CFD], f32, tag=f"y_{q}",
                                    name=f"ybig{q}")
                    ybig.append(yb)
                for j, q, xr in xreps:
                    nc.scalar.activation(
                        ybig[q][:, j * FD:(j + 1) * FD], xr[:], Act.Relu,
                        bias=kb_t[:, j:j + 1], scale=ks_t[:, j:j + 1],
                    )
                # power planes (quarter-stacked full tiles)
                x2 = xtpool.tile([128, FD], f32, tag="x2")
                nc.scalar.activation(x2[:], xt4[:], Act.Square)
                x3 = xtpool.tile([128, FD], f32, tag="x3")
                nc.vector.tensor_tensor(x3[:], x2[:], xt4[:], op=Alu.mult)
                x4 = xtpool.tile([128, FD], f32, tag="x4")
                nc.scalar.activation(x4[:], x2[:], Act.Square)
                # pack per-quarter power block [4 planes x 32 vars, FD]
                pps = []
                for q in range(4):
                    pp = fpool.tile([128, FD], f32, tag=f"pp_{q}")
                    for p, plane in enumerate([xt4, x2, x3, x4]):
                        nc.sync.dma_start(
                            out=pp[p * 32:(p + 1) * 32, :],
                            in_=plane[32 * q:32 * q + 32, :])
                    pps.append(pp)
                # s2 / p3 / p4 per quarter on big tiles
                p3s, p4s = [], []
                for q in range(4):
                    s2 = fpool.tile([128, CFD], f32, tag="s2")
                    nc.scalar.activation(s2[:], ybig[q][:], Act.Square)
                    p3 = fpool.tile([128, CFD], f32, tag=f"p3_{q}")
                    nc.vector.tensor_tensor(p3[:], s2[:], ybig[q][:],
                                            op=Alu.mult)
                    p4 = fpool.tile([128, CFD], f32, tag=f"p4_{q}")
                    if q < p4_act:
                        nc.scalar.activation(p4[:], s2[:], Act.Square)
                    else:
                        nc.gpsimd.tensor_tensor(p4[:], s2[:], s2[:],
                                                op=Alu.mult)
                    p3s.append(p3)
                    p4s.append(p4)
                state[st] = (x_nat, pps, p3s, p4s)

            def tail(st):
                x_nat, pps, p3s, p4s = state.pop(st)
                delta_ps = ps_tail.tile([128, FD], f32, tag="delta")
                for k in range(NCON):
                    wk = wg_t[:, k * V:(k + 1) * V]
                    for q in range(4):
                        if k == 0:
                            rhs = pps[q][:]
                        elif k < 4:
                            rhs = p3s[q][:, (k - 1) * FD:k * FD]
                        else:
                            rhs = p4s[q][:, (k - 4) * FD:(k - 3) * FD]
                        nc.tensor.matmul(
                            delta_ps[32 * q:32 * q + 32, :], wk, rhs,
                            start=(k == 0), stop=(k == NCON - 1),
                            tile_position=(0, 32 * q) if con_coltile else None,
                            skip_group_check=True,
                        )
                dsb = xtpool.tile([128, FD], f32, tag="dsb")
                nc.scalar.copy(dsb[:], delta_ps[:])
                outT_ps = ps_tail.tile([128, FD], f32, tag="outT")
                for g in range(4):
                    nc.tensor.transpose(
                        outT_ps[:, g * 128:(g + 1) * 128],
                        dsb[:, g * 128:(g + 1) * 128],
                        id_t[:],
                    )
                out_sb = iopool.tile([128, FD], f32, tag="out_sb")
                nc.vector.tensor_tensor(out_sb[:], outT_ps[:], x_nat[:],
                                        op=Alu.add)
                nc.sync.dma_start(out=y_re[st], in_=out_sb[:])

            for st in range(min(pipe_ahead + 1, nst)):
                head(st)
            for st in range(nst):
                nxt = st + pipe_ahead + 1
                if nxt < nst:
                    head(nxt)
                tail(st)

    nc.compile()
    _PROG_CACHE[key] = nc
    return nc


def kernel(input, params, _cfg=None):
    from concourse.bass_utils import run_bass_kernel_spmd

    cfg = _cfg or {}
    nc = _build_program(
        p4_act=cfg.get("p4_act", 2),
        rep_rowtile=cfg.get("rep_rowtile", True),
        con_coltile=cfg.get("con_coltile", True),
        pipe_ahead=cfg.get("pipe_ahead", 1),
    )
    WG = _build_weights(params)
    srep, ks, kb, ident = _build_static_consts()
    consts = {"SREP": srep, "KS": ks, "KB": kb, "WG": WG, "IDENT": ident}
    x = np.ascontiguousarray(input, dtype=np.float32)
    in_maps = []
    for i in range(NCORE):
        m = {"input": x[i * NPER:(i + 1) * NPER]}
        m.update(consts)
        in_maps.append(m)
    res = run_bass_kernel_spmd(nc, in_maps, core_ids=list(range(NCORE)),
                               trace=False)
    out = np.concatenate([res.results[i]["output"] for i in range(NCORE)],
                         axis=0)
    if cfg.get("return_results", False):
        return out, res
    return out


# revision 11
# speedup vs baseline: 423.6556x; 423.6556x over previous
# Trainium2 Bass kernel for nn_Decorrelation: out[:,v] = x[:,v] + sum_{c<v} lambda_{v,c}(x_c)*x_c
#
# Math: lambda_{v,c}(x) = B(x) @ params[:, pair(v,c)] with B the 16-fn cubic
# B-spline basis on clamped knots over [-6, 6].  The whole update is recast as
# one feature contraction out = x + G @ W where, per covariate c, the features
#     psi = { x, x^2, x^3, x^4 } + { p3_i = r_i^3 } + { p4_i = r_i^4 },
#     r_i = relu(s_i*(x - e_i))
# (near-side-facing truncated powers, s_i = -1 for e_i<=0 else +1) span
# x*span(B); W = C @ params is built on host from a float64 change-of-basis C.
# Sharding: data-parallel over samples, 16384 rows per core, 8 cores.
#
# Per 2048-sample supertile (per core):
#   DMA in [128,512] -> PE transpose (4x 128x128) to (quarter,var)-major ->
#   DVE clip -> PE replicate knot rows (12 slots/var, 3 chunks, row-tiled
#   across quarters) -> ACT relu -> ACT square -> DVE cube (TT) / ACT+GPSIMD
#   quartic -> power planes x^2..x^4 (ACT/DVE on quarter-stacked tiles,
#   packed per quarter by SBUF DMA) -> PE contract (7 K=128 matmuls per
#   quarter, col-tiled across quarters, fp32 accumulate in PSUM) -> ACT evac
#   -> PE transpose back -> DVE add x -> DMA out.
import numpy as np

V = 32
K = 16
DEG = 3
LO, HI = -5.0, 5.0
SPAN = 0.1 * (HI - LO)
lo, hi = LO - SPAN, HI + SPAN            # -6, 6
NB = K - DEG + 1                          # 14 breakpoints
INNER = np.linspace(lo, hi, NB)
EKNOT = INNER[1:-1]                       # 12 interior knots
KSGN = np.where(EKNOT <= 0, -1.0, 1.0)    # near-side facing
CHI = hi - 1e-6 * (hi - lo)
CLO = lo

N_TOTAL = 131072
NCORE = 8
NPER = N_TOTAL // NCORE                   # 16384
ST = 2048                                 # samples per supertile
FD = 512                                  # free dim of on-chip tiles

NSLOT = 12                                # knot slots, rows r = slot*32 + c
NROW = NSLOT * V                          # 384 rep rows
NCH = 3                                   # rep chunks of 128 rows
NCON = 7                                  # contract matmuls per quarter
CFD = NCH * FD                            # 1536: per-quarter feature free dim


def _bspline_basis(x):
    x = np.asarray(x, np.float64)
    t = np.concatenate([np.full(DEG, lo), INNER, np.full(DEG, hi)])
    x = np.clip(x, lo, hi - 1e-6 * (hi - lo))
    xb = x[:, None]
    B = ((xb >= t[None, :-1]) & (xb < t[None, 1:])).astype(np.float64)
    for r in range(1, DEG + 1):
        tl, tr = t[:-(r + 1)], t[r:-1]
        den1 = tr - tl
        left = np.where(den1 > 0, (xb - tl) / np.where(den1 > 0, den1, 1.0), 0.0)
        tl2, tr2 = t[1:-r], t[r + 1:]
        den2 = tr2 - tl2
        right = np.where(den2 > 0, (tr2 - xb) / np.where(den2 > 0, den2, 1.0), 0.0)
        B = left * B[:, :-1] + right * B[:, 1:]
    return B


_C_CACHE = None


def _fit_C():
    """C [28, 16] float64 s.t. x*B_k(x) = sum_m psi_m(x) * C[m, k],
    psi = [x, x^2, x^3, x^4, r_i^3 (12), r_i^4 (12)]."""
    global _C_CACHE
    if _C_CACHE is not None:
        return _C_CACHE
    g = np.linspace(lo, hi - 1e-5, 8001)
    cols = [g, g * g, g ** 3, g ** 4]
    for e, s in zip(EKNOT, KSGN):
        r = np.maximum(s * (g - e), 0.0)
        cols.append(r ** 3)
    for e, s in zip(EKNOT, KSGN):
        r = np.maximum(s * (g - e), 0.0)
        cols.append(r ** 4)
    Psi = np.stack(cols, axis=1)
    targ = g[:, None] * _bspline_basis(g)
    sc = np.abs(Psi).max(axis=0)
    C, _, _, _ = np.linalg.lstsq(Psi / sc, targ, rcond=None)
    C = C / sc[:, None]
    _C_CACHE = C
    return C


def _build_weights(params):
    """WG [128, 7*32] fp32 (chunk-major in free dim):
       chunk 0   : power features, row p*32+c -> x^(p+1) of var c
       chunks 1-3: r^3 features, row (i%4)*32+c for knot i in chunk i//4
       chunks 4-6: r^4 features, same layout
    """
    C = _fit_C()
    iv, ic = np.tril_indices(V, -1)
    P = np.zeros((K, V, V), np.float64)   # P[k, c, v]
    P[:, ic, iv] = params.astype(np.float64)
    CP = np.einsum("mk,kcv->mcv", C, P)   # [28, c, v]
    WG = np.zeros((NCON, 128, V), np.float64)
    for p in range(4):
        WG[0, p * 32:(p + 1) * 32, :] = CP[p]
    for i in range(12):
        j, sl = divmod(i, 4)
        WG[1 + j, sl * 32:(sl + 1) * 32, :] = CP[4 + i]
        WG[4 + j, sl * 32:(sl + 1) * 32, :] = CP[16 + i]
    WGt = WG.transpose(1, 0, 2).reshape(128, NCON * V)
    return np.ascontiguousarray(WGt, dtype=np.float32)


def _build_static_consts():
    # SREP replicated 4x vertically (rep matmul for quarter q uses
    # partitions 32q..32q+31 for both operands)
    srep1 = np.zeros((V, NROW), np.float32)
    for r in range(NROW):
        srep1[r % 32, r] = 1.0
    srep = np.tile(srep1, (4, 1))
    ks = np.zeros((128, NCH), np.float32)
    kb = np.zeros((128, NCH), np.float32)
    for j in range(NCH):
        for r in range(128):
            i = (r // 32) + 4 * j         # knot index
            ks[r, j] = KSGN[i]
            kb[r, j] = -KSGN[i] * EKNOT[i]
    ident = np.eye(128, dtype=np.float32)
    return srep, ks, kb, ident


_PROG_CACHE = {}


def _build_program(nper=NPER, ncore=NCORE, p4_act=2, rep_rowtile=True,
                   con_coltile=True, pipe_ahead=1, repeat=1):
    """p4_act: how many of the 4 per-quarter p4 (quartic) squares go on ACT;
    the rest run on GPSIMD as s2*s2.  repeat: run the whole supertile loop
    `repeat` times (timing slope measurements)."""
    key = (nper, ncore, p4_act, rep_rowtile, con_coltile, pipe_ahead, repeat)
    if key in _PROG_CACHE:
        return _PROG_CACHE[key]

    import concourse.mybir as mybir
    import concourse.tile as tile
    from concourse import bacc

    f32 = mybir.dt.float32
    Alu = mybir.AluOpType
    Act = mybir.ActivationFunctionType

    nst = nper // ST
    nc = bacc.Bacc("TRN2", target_bir_lowering=False, debug=False,
                   num_devices=ncore)
    x_d = nc.dram_tensor("input", [nper, V], f32, kind="ExternalInput")
    srep_d = nc.dram_tensor("SREP", [128, NROW], f32, kind="ExternalInput")
    ks_d = nc.dram_tensor("KS", [128, NCH], f32, kind="ExternalInput")
    kb_d = nc.dram_tensor("KB", [128, NCH], f32, kind="ExternalInput")
    wg_d = nc.dram_tensor("WG", [128, NCON * V], f32, kind="ExternalInput")
    id_d = nc.dram_tensor("IDENT", [128, 128], f32, kind="ExternalInput")
    y_d = nc.dram_tensor("output", [nper, V], f32, kind="ExternalOutput")

    # x[st*2048 + p*16 + s, v] <-> tile[p, s*32+v] (2KB contiguous per row)
    x_re = x_d.ap().rearrange("(st p s) v -> st p (s v)", p=128, s=16)
    y_re = y_d.ap().rearrange("(st p s) v -> st p (s v)", p=128, s=16)

    with tile.TileContext(nc) as tc:
        with (
            tc.tile_pool(name="const", bufs=1) as cpool,
            tc.tile_pool(name="io", bufs=3) as iopool,
            tc.tile_pool(name="xt", bufs=2) as xtpool,
            tc.tile_pool(name="feat", bufs=2) as fpool,
            tc.tile_pool(name="ps_head", bufs=2, space="PSUM") as ps_head,
            tc.tile_pool(name="ps_rep", bufs=4, space="PSUM") as ps_rep,
            tc.tile_pool(name="ps_tail", bufs=1, space="PSUM") as ps_tail,
        ):
            srep_t = cpool.tile([128, NROW], f32)
            ks_t = cpool.tile([128, NCH], f32)
            kb_t = cpool.tile([128, NCH], f32)
            wg_t = cpool.tile([128, NCON * V], f32)
            id_t = cpool.tile([128, 128], f32)
            for t, d in [(srep_t, srep_d), (ks_t, ks_d), (kb_t, kb_d),
                         (wg_t, wg_d), (id_t, id_d)]:
                nc.sync.dma_start(out=t[:], in_=d.ap())

            state = {}

            def head(st):
                x_nat = iopool.tile([128, FD], f32, tag="x_nat")
                nc.sync.dma_start(out=x_nat[:], in_=x_re[st])
                xt4_ps = ps_head.tile([128, FD], f32, tag="xt4")
                for g in range(4):
                    nc.tensor.transpose(
                        xt4_ps[:, g * 128:(g + 1) * 128],
                        x_nat[:, g * 128:(g + 1) * 128],
                        id_t[:],
                    )
                xt4 = xtpool.tile([128, FD], f32, tag="xt4sb")
                nc.vector.tensor_scalar(xt4[:], xt4_ps[:], CHI, CLO,
                                        op0=Alu.min, op1=Alu.max)
                # replicate knot rows; y written per-chunk into one big tile
                xreps = []
                for j in range(NCH):
                    for q in range(4):
                        xr = ps_rep.tile([128, FD], f32, tag="xrep")
                        nc.tensor.matmul(
                            xr[:],
                            srep_t[32 * q:32 * q + 32, j * 128:(j + 1) * 128],
                            xt4[32 * q:32 * q + 32, :],
                            start=True, stop=True,
                            tile_position=(32 * q, 0) if rep_rowtile else None,
                        )
                        xreps.append((j, q, xr))
                ybig = []
                for q in range(4):
                    yb = fpool.tile([128, CFD], f32, tag=f"y_{q}",
                                    name=f"ybig{q}")
                    ybig.append(yb)
                for j, q, xr in xreps:
                    nc.scalar.activation(
                        ybig[q][:, j * FD:(j + 1) * FD], xr[:], Act.Relu,
                        bias=kb_t[:, j:j + 1], scale=ks_t[:, j:j + 1],
                    )
                # power planes (quarter-stacked full tiles)
                x2 = xtpool.tile([128, FD], f32, tag="x2")
                nc.scalar.activation(x2[:], xt4[:], Act.Square)
                x3 = xtpool.tile([128, FD], f32, tag="x3")
                nc.vector.tensor_tensor(x3[:], x2[:], xt4[:], op=Alu.mult)
                x4 = xtpool.tile([128, FD], f32, tag="x4")
                nc.scalar.activation(x4[:], x2[:], Act.Square)
                # pack per-quarter power block [4 planes x 32 vars, FD]
                pps = []
                for q in range(4):
                    pp = fpool.tile([128, FD], f32, tag=f"pp_{q}")
                    for p, plane in enumerate([xt4, x2, x3, x4]):
                        nc.sync.dma_start(
                            out=pp[p * 32:(p + 1) * 32, :],
                            in_=plane[32 * q:32 * q + 32, :])
                    pps.append(pp)
                # s2 / p3 / p4 per quarter on big tiles
                p3s, p4s = [], []
                for q in range(4):
                    s2 = fpool.tile([128, CFD], f32, tag="s2")
                    nc.scalar.activation(s2[:], ybig[q][:], Act.Square)
                    p3 = fpool.tile([128, CFD], f32, tag=f"p3_{q}")
                    nc.vector.tensor_tensor(p3[:], s2[:], ybig[q][:],
                                            op=Alu.mult)
                    p4 = fpool.tile([128, CFD], f32, tag=f"p4_{q}")
                    if q < p4_act:
                        nc.scalar.activation(p4[:], s2[:], Act.Square)
                    else:
                        nc.gpsimd.tensor_tensor(p4[:], s2[:], s2[:],
                                                op=Alu.mult)
                    p3s.append(p3)
                    p4s.append(p4)
                state[st] = (x_nat, pps, p3s, p4s)

            def tail(st):
                x_nat, pps, p3s, p4s = state.pop(st)
                delta_ps = ps_tail.tile([128, FD], f32, tag="delta")
                for k in range(NCON):
                    wk = wg_t[:, k * V:(k + 1) * V]
                    for q in range(4):
                        if k == 0:
                            rhs = pps[q][:]
                        elif k < 4:
                            rhs = p3s[q][:, (k - 1) * FD:k * FD]
                        else:
                            rhs = p4s[q][:, (k - 4) * FD:(k - 3) * FD]
                        nc.tensor.matmul(
                            delta_ps[32 * q:32 * q + 32, :], wk, rhs,
                            start=(k == 0), stop=(k == NCON - 1),
                            tile_position=(0, 32 * q) if con_coltile else None,
                            skip_group_check=True,
                        )
                dsb = xtpool.tile([128, FD], f32, tag="dsb")
                nc.scalar.copy(dsb[:], delta_ps[:])
                outT_ps = ps_tail.tile([128, FD], f32, tag="outT")
                for g in range(4):
                    nc.tensor.transpose(
                        outT_ps[:, g * 128:(g + 1) * 128],
                        dsb[:, g * 128:(g + 1) * 128],
                        id_t[:],
                    )
                out_sb = iopool.tile([128, FD], f32, tag="out_sb")
                nc.vector.tensor_tensor(out_sb[:], outT_ps[:], x_nat[:],
                                        op=Alu.add)
                nc.sync.dma_start(out=y_re[st], in_=out_sb[:])

            for _rep in range(repeat):
                for st in range(min(pipe_ahead + 1, nst)):
                    head(st)
                for st in range(nst):
                    nxt = st + pipe_ahead + 1
                    if nxt < nst:
                        head(nxt)
                    tail(st)

    nc.compile()
    _PROG_CACHE[key] = nc
    return nc


def kernel(input, params, _cfg=None):
    from concourse.bass_utils import run_bass_kernel_spmd

    cfg = _cfg or {}
    nc = _build_program(
        p4_act=cfg.get("p4_act", 2),
        rep_rowtile=cfg.get("rep_rowtile", True),
        con_coltile=cfg.get("con_coltile", True),
        pipe_ahead=cfg.get("pipe_ahead", 1),
    )
    WG = _build_weights(params)
    srep, ks, kb, ident = _build_static_consts()
    consts = {"SREP": srep, "KS": ks, "KB": kb, "WG": WG, "IDENT": ident}
    x = np.ascontiguousarray(input, dtype=np.float32)
    in_maps = []
    for i in range(NCORE):
        m = {"input": x[i * NPER:(i + 1) * NPER]}
        m.update(consts)
        in_maps.append(m)
    res = run_bass_kernel_spmd(nc, in_maps, core_ids=list(range(NCORE)),
                               trace=False)
    out = np.concatenate([res.results[i]["output"] for i in range(NCORE)],
                         axis=0)
    if cfg.get("return_results", False):
        return out, res
    return out
